# revision 126
# baseline (speedup 1.0000x reference)
"""Trainium2 Bass kernel for nn_BlockwiseAttention (sparse_attention).

Full (unsharded) inputs in, full output out.  Internally shards across the
8 NeuronCores as (batch x head-group): core c -> batch c//4, heads
[4*(c%4), 4*(c%4)+4).  Each core computes qkv projection + RoPE + masked
attention + its slice of the output projection; the host sums the per-core
(and per head-pair) partial projections.

v3 design (~181us cost-model; both PE and ACT near saturation):
  - all matmuls bf16; scores^T tiles [t=128, s=1024] in PSUM; exp on ACT
    (1038ns/tile -> the 128-tile exp chain is the steady-state cadence)
  - mask applied POST-exp as zeroing (the reference keeps ~allowed, i.e.
    the COMPLEMENT of the causal band -- attention is ~92% dense)
  - AV out[s,d] with a ones-column folded into v_all (65-col rhs windows)
    so the softmax denominators ride the same matmuls: halves the PE
    instruction count vs separate ones-matmuls; av splits into two
    psum banks (4 x 65 cols each, a 520-col tile would cross a bank)
  - AV batches deferred ~6 slots on a CROSS-LOOP queue so loop
    boundaries never stall the next QK/exp chain behind a drain burst
  - weights host-prearranged for 2KB DMA descriptors (256B descriptors
    run at half rail rate); rail order: xt -> wb -> rope tables -> rest;
    k01-hi projections fill the dead PE window before the first QK
  - prologue rope: ACT evacuates psum, Pool takes the cos-multiplies of
    the late pieces (it is desc-gen-bound until ~21us), DVE runs
    shuffle+mul+add; 4-deep rope pools avoid WAR serialization
  - multi-wait instructions peel extra waits onto SEQ nops sorted so the
    latest-satisfied wait stays on the instruction itself
  - out_proj per head-pair; pair-0/pair-1-lo interleaved into the
    attention slots, the 20-item tail drains through 3 psum banks with
    ACT/DVE alternating evacs and paired [128,1024] output DMAs
"""

import sys
import numpy as np
from collections import deque

for _p in ("/opt/trn_rl_repo",):
    if _p not in sys.path:
        sys.path.insert(0, _p)

import ml_dtypes

import concourse.bass as bass
import concourse.tile as tile
from concourse import mybir
from concourse.bass import ts, ds
from concourse.bass_utils import run_bass_kernel_spmd

# ---------------------------------------------------------------- constants
B, S, E = 2, 2048, 1024
H, D = 16, 64
HL = 4                     # heads per core
N_CORES = 8
BLOCK, NG, WIN = 256, 4, 128
ROPE_BASE = 10000.0

F32 = mybir.dt.float32
BF16 = mybir.dt.bfloat16
EXP = mybir.ActivationFunctionType.Exp

NT = S // 128              # 16 t-chunks
_SHUF = [i ^ 1 for i in range(32)]


# ---------------------------------------------------------------- program
def build_nc(for_sim: bool = False) -> bass.Bass:
    nc = bass.Bass()

    xt = nc.declare_dram_parameter("xt", [E, S], BF16, isOutput=False)
    wqkvp = nc.declare_dram_parameter("wqkvp", [128, 4096], BF16,
                                      isOutput=False)
    wvd = nc.declare_dram_parameter("wvd", [E, 256], BF16, isOutput=False)
    woutp = nc.declare_dram_parameter("woutp", [128, 2048], BF16,
                                      isOutput=False)
    cosb = nc.declare_dram_parameter("cosb", [128, S], BF16, isOutput=False)
    sinb = nc.declare_dram_parameter("sinb", [128, S], BF16, isOutput=False)
    trid2 = nc.declare_dram_parameter("trid2", [128, 384], BF16,
                                      isOutput=False)
    t0row = nc.declare_dram_parameter("t0row", [4, S], BF16, isOutput=False)
    yt2 = nc.declare_dram_parameter("yt2", [2, E, S], BF16, isOutput=True)

    with tile.TileContext(nc) as tc:
        _body(nc, tc, xt, wqkvp, wvd, woutp, cosb, sinb, trid2, t0row, yt2)
    if not for_sim:
        _split_waits(nc)
    return nc


def _split_waits(nc):
    """walrus's per-instruction sync structs accept few sync waits; peel
    extra waits onto same-engine ENGINE_NOPs inserted right before the
    instruction (the sequencer processes their waits first)."""
    eng = mybir.EngineType
    builders = {eng.PE: nc.tensor, eng.DVE: nc.vector,
                eng.Activation: nc.scalar, eng.Pool: nc.gpsimd,
                eng.SP: nc.sync}
    fn = nc.m.functions[0]

    def mk_nop(builder):
        builder.nop()
        scratch = fn.blocks[-1].instructions
        raw = scratch.pop()
        assert type(raw).__name__ == "InstNoOp", type(raw).__name__
        return raw

    for blk in fn.blocks:
        out = []
        changed = False
        for inst in blk.instructions:
            si = inst.sync_info
            if (si is not None and len(si.on_wait) > 1
                    and inst.engine in builders):
                # keep the LATEST-satisfied wait on the instruction itself:
                # earlier sems (smaller wait_value) go to the peeled nops,
                # which the sequencer processes first and finds satisfied
                def _wkey(w):
                    v = w.wait_value
                    return v if isinstance(v, int) else -1
                waits = sorted(si.on_wait, key=_wkey)
                for w in waits[:-1]:
                    nop = mk_nop(builders[inst.engine])
                    nop.engine = inst.engine
                    nop.sync_info = mybir.SyncInfo(on_wait=[w], on_update=[])
                    out.append(nop)
                si.on_wait = [waits[-1]]
                changed = True
            out.append(inst)
        if changed:
            blk.instructions[:] = out
    return nc


def _body(nc, tc, xt, wqkvp, wvd, woutp, cosb, sinb, trid2, t0row, yt2):
    from contextlib import ExitStack

    with ExitStack() as ctx:
        # ---------------- SBUF pools -------------------------------------
        consts = ctx.enter_context(tc.tile_pool(name="consts", bufs=1))
        xt_p = ctx.enter_context(tc.tile_pool(name="xt_p", bufs=1))
        w_p = ctx.enter_context(tc.tile_pool(name="w_p", bufs=1))
        qk_sb = ctx.enter_context(tc.tile_pool(name="qk_sb", bufs=1))
        v_sb = ctx.enter_context(tc.tile_pool(name="v_sb", bufs=1))
        rope_p = ctx.enter_context(tc.tile_pool(name="rope_p", bufs=4))
        unn_p = ctx.enter_context(tc.tile_pool(name="unn_p", bufs=11))
        osb_p = ctx.enter_context(tc.tile_pool(name="osb_p", bufs=8))
        rec_p = ctx.enter_context(tc.tile_pool(name="rec_p", bufs=3))
        oT_p = ctx.enter_context(tc.tile_pool(name="oT_p", bufs=1))
        ysb_p = ctx.enter_context(tc.tile_pool(name="ysb_p", bufs=24))

        # ---------------- PSUM pools (16KB budget, creation order) -------
        qk_ps = ctx.enter_context(
            tc.tile_pool(name="qk_ps", bufs=2, space="PSUM"))   # 8KB
        av_ps = ctx.enter_context(
            tc.tile_pool(name="av_ps", bufs=1, space="PSUM"))   # 2KB
        avb_ps = ctx.enter_context(
            tc.tile_pool(name="avb_ps", bufs=1, space="PSUM"))  # 2KB
        y_ps = ctx.enter_context(
            tc.tile_pool(name="y_ps", bufs=1, space="PSUM"))    # 2KB
        tr_ps = ctx.enter_context(
            tc.tile_pool(name="tr_ps", bufs=1, space="PSUM"))   # 1 bank

        # ---------------- SBUF tiles -------------------------------------
        cos_t = consts.tile([128, S], BF16, tag="cos")
        sin_t = consts.tile([128, S], BF16, tag="sin")
        timask = consts.tile([128, 384], BF16, tag="timask")
        tri_t = timask[:, 0:256]
        ident_t = timask[:, 256:384]
        t0_t = consts.tile([4, S], BF16, tag="t0")
        wout2 = consts.tile([128, 2048], BF16, tag="wout2")
        wout_t = [wout2[:, 0:1024], wout2[:, 1024:2048]]

        xt_t = [xt_p.tile([128, S], BF16, tag=f"xt{k}", name=f"xt{k}")
                for k in range(8)]
        # wb[m]: m0=q01 m1=q23 m2=k01 m3=k23 ([128, 8 e-chunks x 128 cols])
        # wb1/wb3 share one tile (and one late DMA: each extra SWDGE
        # transfer costs ~1us of Pool descriptor generation)
        wb0_t = w_p.tile([128, 1024], BF16, tag="wb0")
        wb2_t = w_p.tile([128, 1024], BF16, tag="wb2")
        wb13 = w_p.tile([128, 2048], BF16, tag="wb13")
        wb = [wb0_t, wb13[:, 0:1024], wb2_t, wb13[:, 1024:2048]]
        wv = w_p.tile([128, 2048], BF16, tag="wv")
        qkT = [qk_sb.tile([128, S], BF16, tag=f"qkT{m}", name=f"qkT{m}")
               for m in range(4)]
        # v_all block (T, h) at col 260*T + 65*h: 64 v cols + a ones col
        # (the ones col turns the separate denominator matmuls into one
        # extra AV column -- halves the PE instruction count)
        v_all = v_sb.tile([128, NT * 260], BF16, tag="vall")
        oT2 = [oT_p.tile([128, S], BF16, tag=f"oT{p}", name=f"oT{p}")
               for p in range(2)]

        # ---------------- DMA queues -------------------------------------
        # One shared DMA data rail (~1.5us per 512KB; sub-512B descriptors
        # run at HALF rate -- the wb blocks are host-prearranged so every
        # transfer moves 2KB rows).  xt leads the rail; wb0/wb2 (needed
        # with the first xt chunks) follow on their own queues; the rope
        # tables land just before xt7 so the prologue rope chain starts
        # the moment q01 stops.
        for k in (0, 1, 2, 3, 4):
            nc.sync.dma_start(out=xt_t[k][:], in_=xt[ts(k, 128), :])
        nc.gpsimd.dma_start(out=wb[0], in_=wqkvp[:, 0:1024])
        nc.scalar.dma_start(out=wb[2], in_=wqkvp[:, 1024:2048])
        for k in (5, 6, 7):
            nc.gpsimd.dma_start(out=xt_t[k][:], in_=xt[ts(k, 128), :])
        nc.gpsimd.dma_start(out=cos_t[:, 0:1024], in_=cosb[:, 0:1024])
        nc.gpsimd.dma_start(out=sin_t[:, 0:1024], in_=sinb[:, 0:1024])
        nc.gpsimd.dma_start(out=cos_t[:, 1024:S], in_=cosb[:, 1024:S])
        nc.gpsimd.dma_start(out=sin_t[:, 1024:S], in_=sinb[:, 1024:S])
        nc.gpsimd.dma_start(
            out=wv[:].rearrange("p (k c) -> p k c", k=8),
            in_=wvd[:].rearrange("(k p) c -> p k c", p=128))
        nc.gpsimd.dma_start(out=timask[:], in_=trid2[:])
        nc.gpsimd.dma_start(out=t0_t[:], in_=t0row[:])
        # late weights at the BACK of the Pool-paced queue: their
        # descriptor generation overlaps the xt stream (Pool is idle then)
        # and the rail FIFO still serves them after the rope tables
        nc.gpsimd.dma_start(out=wb13[:], in_=wqkvp[:, 2048:4096])
        nc.gpsimd.dma_start(out=wout2[:], in_=woutp[:])

        # dummy-matmul source first in the DVE stream (PE p-state anchor)
        dumsrc = consts.tile([128, 512], BF16, tag="dumsrc")
        nc.vector.memset(dumsrc[:], 0.0)
        # ones columns of v_all (every 65th col)
        nc.vector.memset(
            v_all[:].rearrange("p (n k) -> p n k", k=65)[:, :, 64], 1.0)

        # xt arrival order on the shared rail
        KORD = [0, 1, 5, 2, 6, 3, 4, 7]

        # ---------------- helpers ----------------------------------------
        def rope_piece(m, src_ap, gcol, copy_eng=None, pool_mula=False):
            """qkT[m][:, gcol:gcol+512] = src*cos + shuffle(src)*sin.

            The psum piece is evacuated to bf16 first (frees the psum WAR
            fast; GPSIMD cannot touch PSUM, so this is ACT in the prologue
            and DVE elsewhere); the arithmetic runs on DVE in all-bf16
            SBUF (fast 2x/4x modes)."""
            sl = ds(gcol, 512)
            qb = rope_p.tile([128, 512], BF16, tag="qb",
                             name=f"qb{m}_{gcol}")
            if copy_eng is nc.scalar:
                nc.scalar.copy(qb[:], src_ap)
            else:
                nc.vector.tensor_copy(qb[:], src_ap)
            a = rope_p.tile([128, 512], BF16, tag="ra",
                            name=f"ra{m}_{gcol}")
            rs = rope_p.tile([128, 512], BF16, tag="rs",
                             name=f"rs{m}_{gcol}")
            b = rope_p.tile([128, 512], BF16, tag="rb",
                            name=f"rb{m}_{gcol}")
            # pool_mula pieces: the cos-multiply runs on the Pool engine in
            # parallel with the DVE shuffle+mul chain.  Pool is busy with
            # SWDGE descriptor generation until ~21us (ring-paced by the
            # DMA rail), so only the non-critical prologue pieces use it.
            mul_a = (nc.gpsimd.tensor_mul if pool_mula
                     else nc.vector.tensor_mul)
            mul_a(a[:], qb[:], cos_t[:, sl])
            nc.vector.stream_shuffle(rs[:], qb[:], _SHUF)
            nc.vector.tensor_mul(b[:], rs[:], sin_t[:, sl])
            nc.vector.tensor_add(qkT[m][:, sl], a[:], b[:])

        # ---------------- prologue: q01, k01, v(T0-3) --------------------
        # q01 into two [128,1024] psum halves, k-loop interleaved
        # dummy matmuls keep the PE busy-period alive from t~0 while the xt
        # DMAs stream in: instruction cost is priced at dispatch with the
        # ramp known then, so a burst released after a stall is priced at
        # the LOW p-state.
        dum = y_ps.tile([128, 512], F32, tag="y", name="dum")

        def dummy():
            nc.tensor.matmul(dum[:], dumsrc[:, 0:128], dumsrc[:],
                             start=True, stop=True)

        # q01 + k01-lo interleaved in one chunk-chasing loop: q01 fills the
        # two qk-pool tiles (lo first: QK-T0 lands on b0, WARing only the
        # early q-lo ropes); k01-lo accumulates in the av/dn banks
        psq_lo = qk_ps.tile([128, 1024], F32, tag="qk", name="psq_lo")
        psq_hi = qk_ps.tile([128, 1024], F32, tag="qk", name="psq_hi")
        ka = av_ps.tile([128, 512], F32, tag="av", name="k01a")
        kb = avb_ps.tile([128, 512], F32, tag="avb", name="k01b")
        for _ in range(4):
            dummy()
        for ki, k in enumerate(KORD):
            for hi, pp in ((0, psq_lo), (1, psq_hi)):
                for j in range(2):
                    nc.tensor.matmul(
                        pp[:, ts(j, 512)],
                        wb[0][:, ts(k, 128)],
                        xt_t[k][:, ds(1024 * hi + 512 * j, 512)],
                        start=(ki == 0), stop=(ki == 7))
            nc.tensor.matmul(ka[:], wb[2][:, ts(k, 128)],
                             xt_t[k][:, ds(0, 512)],
                             start=(ki == 0), stop=(ki == 7))
            nc.tensor.matmul(kb[:], wb[2][:, ts(k, 128)],
                             xt_t[k][:, ds(512, 512)],
                             start=(ki == 0), stop=(ki == 7))

        # rope order: q-lo pieces unblock QK-T0's WAR; k-p0 is its data
        # dep.  ACT (idle before the first exp) does the psum-evac copies.
        for _ in range(8):
            dummy()
        rope_piece(0, psq_lo[:, ts(0, 512)], 0, copy_eng=nc.scalar)
        rope_piece(0, psq_lo[:, ts(1, 512)], 512, copy_eng=nc.scalar)
        rope_piece(2, ka[:], 0, copy_eng=nc.scalar, pool_mula=True)
        rope_piece(2, kb[:], 512, copy_eng=nc.scalar, pool_mula=True)
        rope_piece(0, psq_hi[:, ts(0, 512)], 1024, copy_eng=nc.scalar,
                   pool_mula=True)
        rope_piece(0, psq_hi[:, ts(1, 512)], 1536, copy_eng=nc.scalar,
                   pool_mula=True)


        # v(T0..T3) is emitted as slot items at the start of h0-sh0 (wv
        # lands after the xt stream; attention QK/exp starts first)


        # ---------------- interleaved work queues ------------------------
        # v split by head-pair: heads 0/1 during h0-sh0 (needed first),
        # heads 2/3 during the otherwise-idle h1-sh1 window
        vq0 = deque((T0, 0) for T0 in range(0, NT, 2))
        vq1 = deque((T0, 1) for T0 in range(0, NT, 2))
        def kh_item(p, pool, tag):
            yp = pool.tile([128, 512], F32, tag=tag, name=f"kh{p}")
            for k in range(8):
                nc.tensor.matmul(yp[:], wb[2][:, ts(k, 128)],
                                 xt_t[k][:, ds(512 * p, 512)],
                                 start=(k == 0), stop=(k == 7))
            rope_piece(2, yp[:], 512 * p, pool_mula=True)

        # k01-hi projections run in the early h0-sh0 slots (T1/T3): those
        # slots have no AV batches yet (6-deep deferral), so the exp
        # cadence absorbs the bursts, and the pre-QK PE stream -- now the
        # prologue critical path -- stays 3.4us shorter.  Their psum lives
        # in the av/avb banks, free until the first AV write at slot 6.
        khq = deque([(2, av_ps, "av"), (3, avb_ps, "avb")])

        def v_item(T0, half):
            # v projections (2 T-chunks x 2 heads) share the y-psum bank
            vs = y_ps.tile([128, 512], F32, tag="y", name=f"vs{T0}_{half}")
            for t2 in range(2):
                T = T0 + t2
                for k in range(8):
                    nc.tensor.matmul(vs[:, ds(256 * t2, 128)],
                                     xt_t[k][:, ts(T, 128)],
                                     wv[:, ds(256 * k + 128 * half, 128)],
                                     start=(k == 0), stop=(k == 7))
                # scatter the 2 heads into the 65-stride v_all layout
                dst = v_all[:, ds(260 * T + 130 * half, 130)] \
                    .rearrange("p (h k) -> p h k", k=65)[:, :, 0:64]
                src = vs[:, ds(256 * t2, 128)] \
                    .rearrange("p (h k) -> p h k", k=64)
                nc.vector.tensor_copy(dst, src)

        # q23/k23 projection pieces through the y-psum bank
        projmms = deque()

        def _mk_proj(m, p):
            yp = y_ps.tile([128, 512], F32, tag="y", name=f"pj{m}_{p}")

            def mk_mm(k):
                def f():
                    nc.tensor.matmul(yp[:], wb[m][:, ts(k, 128)],
                                     xt_t[k][:, ds(512 * p, 512)],
                                     start=(k == 0), stop=(k == 7))
                    if k == 7:
                        rope_piece(m, yp[:], 512 * p)
                return f
            return [mk_mm(k) for k in range(8)]

        projq = deque((m, p) for m in (1, 3) for p in range(4))

        def pump_proj(n=2):
            if not projmms and projq:
                projmms.extend(_mk_proj(*projq.popleft()))
            for _ in range(n):
                if projmms:
                    projmms.popleft()()

        # out_proj items
        yq0 = deque((0, e, sc) for sc in range(4) for e in range(8))
        yq1a = deque((1, e, sc) for sc in range(2) for e in range(8))
        yq1b = deque((1, e, sc) for sc in (2, 3) for e in range(8))
        ycnt = [0]

        # y evacuations batch in [128,1024] staging tiles: one DMA per two
        # out_proj tiles (the HWDGE per-transfer overhead is ~625ns)
        ybuf = {}

        def y_item(pair, e, sc, pool=None, tag="y", act_evac=False,
                   by_e=False, dma_eng=None, single=False):
            ycnt[0] += 1
            yp = (pool or y_ps).tile([128, 512], F32, tag=tag,
                                     name=f"y{pair}_{e}_{sc}")
            nc.tensor.matmul(yp[:], wout_t[pair][:, ts(e, 128)],
                             oT2[pair][:, ts(sc, 512)],
                             start=True, stop=True)
            if single:
                # tail: per-item DMA so the last transfers are not gated
                # on a pair partner finishing
                ysb = ysb_p.tile([128, 512], BF16, tag="ysb1", bufs=8,
                                 name=f"ysb1_{pair}_{e}_{sc}")
                if act_evac:
                    nc.scalar.copy(ysb[:], yp[:])
                else:
                    nc.vector.tensor_copy(ysb[:], yp[:])
                (dma_eng or nc.sync).dma_start(
                    out=yt2[pair, ts(e, 128), ts(sc, 512)], in_=ysb[:])
                return
            key = (pair, e // 2, sc) if by_e else (pair, e, sc // 2)
            slot = (e % 2) if by_e else (sc % 2)
            if key not in ybuf:
                ybuf[key] = [ysb_p.tile([128, 1024], BF16, tag="ysb",
                                        name=f"ysb{key[0]}_{key[1]}_{key[2]}"
                                        f"{'e' if by_e else 's'}"),
                             0]
            ent = ybuf[key]
            dst = ent[0][:, ds(512 * slot, 512)]
            if act_evac:
                nc.scalar.copy(dst, yp[:])
            else:
                nc.vector.tensor_copy(dst, yp[:])
            ent[1] += 1
            if ent[1] == 2:
                de = dma_eng or nc.sync
                if by_e:
                    de.dma_start(
                        out=yt2[pair, ds(256 * (e // 2), 256), ts(sc, 512)]
                        .rearrange("(two p) c -> p two c", p=128),
                        in_=ent[0][:].rearrange("p (two c) -> p two c",
                                                two=2))
                else:
                    de.dma_start(
                        out=yt2[pair, ts(e, 128),
                                ds(1024 * (sc // 2), 1024)],
                        in_=ent[0][:])
                del ybuf[key]

        # ---------------- attention --------------------------------------
        def emit_qk(h, shi, T, ps):
            qq = qkT[h // 2]
            kk = qkT[2 + h // 2]
            r0 = 64 * (h % 2)
            for j in range(2):
                nc.tensor.matmul(
                    ps[:, ts(j, 512)],
                    kk[r0:r0 + 64, ts(T, 128)],
                    qq[r0:r0 + 64, ds(1024 * shi + 512 * j, 512)],
                    start=True, stop=True)

        def emit_bands(shi, T, unn):
            lo = 1024 * shi
            b0 = 128 * T
            if lo <= b0 < lo + 1024:
                c = b0 - lo
                nc.gpsimd.tensor_mul(unn[:, ds(c, 128)], unn[:, ds(c, 128)],
                                     tri_t[:, 0:128])
            b1 = b0 + 128
            if b1 < S and lo <= b1 < lo + 1024:
                c = b1 - lo
                if T % 2 == 0:
                    nc.gpsimd.memset(unn[:, ds(c, 128)], 0.0)
                else:
                    nc.gpsimd.tensor_mul(unn[:, ds(c, 128)],
                                         unn[:, ds(c, 128)],
                                         tri_t[:, 128:256])
            if T == 0:
                nc.vector.tensor_mul(unn[0:4, :], unn[0:4, :],
                                     t0_t[:, ds(lo, 1024)])

        def emit_avdn(h, T, unn, ava, avb):
            # out chunk c = [av 64 | den 1]: the ones col of v_all folds the
            # denominator into the same matmul.  ava holds chunks 0-3, avb
            # 4-7 (one psum bank each; a 520-col tile would cross a bank).
            first, last = (T == 0), (T == NT - 1)
            rhs = v_all[:, ds(260 * T + 65 * h, 65)]
            for c in range(8):
                t_, cc = (ava, c) if c < 4 else (avb, c - 4)
                nc.tensor.matmul(t_[:, ds(65 * cc, 65)], unn[:, ts(c, 128)],
                                 rhs,
                                 start=(first and c in (0, 4)),
                                 stop=(last and c in (3, 7)))

        def make_norm(h, shi, ava, avb):
            """Split norm: DVE part (recip + scalar-muls into osb tiles)
            emitted at the next loop's T1; the PE transposes + oT2 copies
            are pumped as two thunks at T2/T3 so they don't sit between
            consecutive QK matmuls while DVE catches up."""
            p = h // 2
            r0 = 64 * (h % 2)
            osbs = []

            def dve_part():
                rec = rec_p.tile([128, 8], F32, tag="rec",
                                 name=f"rec{h}_{shi}")
                nc.vector.reciprocal(
                    rec[:, 0:4],
                    ava[:].rearrange("p (n k) -> p n k", k=65)[:, :, 64])
                nc.vector.reciprocal(
                    rec[:, 4:8],
                    avb[:].rearrange("p (n k) -> p n k", k=65)[:, :, 64])
                for c in range(8):
                    t_, cc = (ava, c) if c < 4 else (avb, c - 4)
                    osb = osb_p.tile([128, 64], BF16, tag="osb",
                                     name=f"o{h}_{shi}_{c}")
                    nc.vector.tensor_scalar_mul(
                        osb[:], t_[:, ds(65 * cc, 64)], rec[:, c:c + 1])
                    osbs.append(osb)

            def mk_quad(q):
                def quad():
                    tr = tr_ps.tile([64, 512], BF16, tag="tr",
                                    name=f"tr{h}_{shi}_{q}")
                    for j in range(4):
                        nc.tensor.transpose(tr[:, ds(128 * j, 128)],
                                            osbs[4 * q + j][:], ident_t)
                    nc.vector.tensor_copy(
                        oT2[p][r0:r0 + 64, ds(1024 * shi + 512 * q, 512)],
                        tr[:])
                return quad
            return dve_part, deque([mk_quad(0), mk_quad(1)])

        prev_dve = [None]
        tr_thunks = deque()

        def pump(h, shi, T):
            g = (h * 2 + shi) * 16 + T
            did_quad = False
            tq = 10 if (h == 3 and shi == 1) else 12
            if tr_thunks and T >= tq:
                tr_thunks.popleft()()
                did_quad = True
            if 16 <= g < 56 and not did_quad:
                pump_proj()
            elif g >= 64:
                # keep DVE free around loop boundaries (the norm chain
                # must not queue behind y evacuations)
                if yq0 and g >= 66 and 2 <= T <= 14:
                    y_item(*yq0.popleft())
                elif yq1a and g >= 122:
                    y_item(*yq1a.popleft())
                    if yq1a:
                        y_item(*yq1a.popleft())

        deferred = []          # cross-loop AV queue: (h, T, unn, ava, avb)

        def attn(h, shi):
            ava = av_ps.tile([128, 260], F32, tag="av", name=f"av{h}_{shi}")
            avb = avb_ps.tile([128, 260], F32, tag="avb",
                              name=f"avb{h}_{shi}")
            last = (h == 3 and shi == 1)
            for T in range(NT):
                ps = qk_ps.tile([128, 1024], F32, tag="qk",
                                name=f"qk{h}_{shi}_{T}")
                emit_qk(h, shi, T, ps)
                unn = unn_p.tile([128, 1024], BF16, tag="unn",
                                 name=f"unn{h}_{shi}_{T}")
                nc.scalar.activation(unn[:], ps[:], EXP, scale=0.125)
                # norm of the PREVIOUS loop: only after its last AV batch
                # has drained from the cross-loop queue (slot 6)
                if T == 7 and prev_dve[0] is not None:
                    prev_dve[0]()
                    prev_dve[0] = None
                emit_bands(shi, T, unn)
                # v/k-hi projections must precede their consumers in PE
                # order; v leads its AV consumer by the deferral depth
                if h == 0 and shi == 0:
                    if khq and T in (1, 3):
                        kh_item(*khq.popleft())
                    elif vq0:
                        v_item(*vq0.popleft())
                elif h == 1 and shi == 1 and vq1 and T >= 8:
                    v_item(*vq1.popleft())
                # AV deferred ~6 slots (carried across loop boundaries) so
                # PE never sits on a cross-engine dependency between
                # consecutive QK matmuls and the next loop's QK/exp chain
                # is not stalled behind an end-of-loop drain burst
                deferred.append((h, T, unn, ava, avb))
                if len(deferred) > 6:
                    emit_avdn(*deferred.pop(0))
                    # last loop: drain the deferral early so the tail's
                    # norm chain is not stuck behind 6 AV flush batches
                    if last and T >= 7 and deferred:
                        emit_avdn(*deferred.pop(0))
                pump(h, shi, T)
            if last:
                while deferred:
                    emit_avdn(*deferred.pop(0))
            dve_part, quads = make_norm(h, shi, ava, avb)
            prev_dve[0] = dve_part
            tr_thunks.extend(quads)

        for h in range(4):
            for shi in range(2):
                attn(h, shi)

        # ---------------- tail: remaining out_proj -----------------------
        # keep the PE busy-period alive through the norm/transpose chain so
        # the tail out_proj matmuls are priced at the full p-state (the
        # qk psum pool is free once the last exp has drained)
        dumt = qk_ps.tile([128, 1024], F32, tag="qk", name="dumt")

        def tdummy(n):
            for _ in range(n):
                nc.tensor.matmul(dumt[:, 0:512], dumsrc[:, 0:128],
                                 dumsrc[:], start=True, stop=True)

        pools = [(y_ps, "y"), (av_ps, "av"), (avb_ps, "avb")]
        cnt = [0]

        def drain(items, by_e=True, act_all=False, act_mod=2,
                  single=False, dma=None):
            for pair, e, sc in items:
                pool, tag = pools[cnt[0] % 3]
                # engine per staging PAIR (a shared tile's WAW ordering
                # would serialize mixed engines); ACT takes the larger
                # share (DVE also runs the tail norm + oT copies)
                gi = cnt[0] if single else cnt[0] // 2
                y_item(pair, e, sc, pool=pool, tag=tag,
                       act_evac=(act_all or gi % act_mod != 0),
                       by_e=by_e,
                       dma_eng=dma, single=single)
                cnt[0] += 1

        # leftover yq1a items first: their oT2 columns are long ready and
        # ACT idles right after the last exp; DVE runs the last norm chain
        # in parallel
        leftover = list(yq1a)
        yq1a.clear()
        yq1b.clear()
        drain(leftover, by_e=False, act_all=True)
        prev_dve[0]()
        prev_dve[0] = None
        tdummy(8)
        tr_thunks.popleft()()      # quad0 -> oT2[1] sh1 cols sc2
        drain([(1, e, 2) for e in range(4)])
        tr_thunks.popleft()()      # quad1 -> sc3
        drain([(1, e, 2) for e in range(4, 8)])
        drain([(1, e, 3) for e in range(8)])


# ---------------------------------------------------------------- host side
def _host_consts():
    w_pos = np.arange(S, dtype=np.float64)
    inv_freq = 1.0 / (ROPE_BASE ** (np.arange(0, D, 2, dtype=np.float64) / D))
    freqs = np.outer(w_pos, inv_freq)                    # [S, 32]
    emb = np.concatenate([freqs, freqs], axis=-1)        # [S, 64]
    cosT = np.cos(emb).T                                 # [64, S]
    sinT = np.sin(emb).T
    # shuffled operand is q[d^1]; rot(q)[d] = sign(d) * q[d^1], sign=-1 on
    # even d -> fold into sin
    sgn = np.where(np.arange(D) % 2 == 0, -1.0, 1.0)[:, None]
    sinT = sinT * sgn
    cosb = np.concatenate([cosT, cosT], 0).astype(ml_dtypes.bfloat16)
    sinb = np.concatenate([sinT, sinT], 0).astype(ml_dtypes.bfloat16)

    tl = np.arange(128)[:, None]
    so = np.arange(128)[None, :]
    tri = np.zeros((128, 256), np.float32)
    tri[:, 0:128] = (so < tl)          # TRI_UP: keep s < t
    tri[:, 128:256] = (so > tl)        # TRI_LO: keep s > t
    tri = tri.astype(ml_dtypes.bfloat16)

    t0 = (np.arange(S)[None, :] < np.arange(4)[:, None]) \
        .astype(ml_dtypes.bfloat16)    # keep s < t for global rows
    ident = np.eye(128, dtype=ml_dtypes.bfloat16)
    return cosb, sinb, tri, t0, ident


def core_inputs(x, w_qkv, w_out, c):
    cosb, sinb, tri, t0, ident = _host_consts()
    wq3 = w_qkv.reshape(E, 3, H, D)
    b, g = divmod(c, HL)
    h0 = HL * g
    wq = wq3[:, 0, h0:h0 + HL].reshape(E, 256)
    wk = wq3[:, 1, h0:h0 + HL].reshape(E, 256)
    wv = wq3[:, 2, h0:h0 + HL].reshape(E, 256)
    # wb[m][p, 128k + c] = w_m[128k + p, c]: prearranged so the device DMA
    # moves contiguous 2KB rows (256B descriptors run at half DMA rate);
    # column order [q01 | k01 | q23 | k23] so the early (m0, m2) and late
    # (m1, m3) blocks are each one contiguous transfer
    wqkvp = np.zeros((128, 4096), np.float32)
    for i, wm in enumerate((wq[:, 0:128], wk[:, 0:128],
                            wq[:, 128:256], wk[:, 128:256])):
        wqkvp[:, 1024 * i:1024 * (i + 1)] = \
            wm.reshape(8, 128, 128).transpose(1, 0, 2).reshape(128, 1024)
    return {
        "xt": np.ascontiguousarray(x[b].T).astype(ml_dtypes.bfloat16),
        "wqkvp": wqkvp.astype(ml_dtypes.bfloat16),
        "wvd": np.ascontiguousarray(wv).astype(ml_dtypes.bfloat16),
        "woutp": np.concatenate(
            [w_out[h0 * D:h0 * D + 128, :],
             w_out[h0 * D + 128:h0 * D + 256, :]],
            axis=1).astype(ml_dtypes.bfloat16),
        "cosb": cosb, "sinb": sinb,
        "trid2": np.concatenate([tri, ident], axis=1),
        "t0row": t0,
    }


_NC_CACHE = None
LAST_RUN = None


def kernel(x, w_qkv, w_out):
    global _NC_CACHE, LAST_RUN
    x = np.asarray(x, np.float32)
    w_qkv = np.asarray(w_qkv, np.float32)
    w_out = np.asarray(w_out, np.float32)

    if _NC_CACHE is None:
        _NC_CACHE = build_nc()
    nc = _NC_CACHE

    in_maps = [core_inputs(x, w_qkv, w_out, c) for c in range(N_CORES)]
    res = run_bass_kernel_spmd(nc, in_maps, list(range(N_CORES)))
    LAST_RUN = res
    outs = res.results

    y = np.zeros((B, S, E), np.float32)
    for c in range(N_CORES):
        b = c // HL
        yt = np.asarray(outs[c]["yt2"], dtype=np.float32)
        y[b] += yt[0].T
        y[b] += yt[1].T
    return y


if __name__ == "__main__":
    rng = np.random.default_rng(0)
    x = rng.standard_normal((B, S, E), dtype=np.float32)
    wq = (rng.standard_normal((E, 3 * E), dtype=np.float32) * E ** -0.5)
    wo = (rng.standard_normal((E, E), dtype=np.float32) * E ** -0.5)
    out = kernel(x, wq, wo)
    print(out.shape, out.dtype, np.abs(out).mean())

